# revision 15
# baseline (speedup 1.0000x reference)
"""BitLinear on 8 Trainium2 NeuronCores, column-parallel over out_features.

v5 — host-side weight scale; single weight pass on device.

scale_w = mean(|weight|) is a pure function of the (static) weight matrix, so
kernel() computes it on the host (float64 accumulate, like a deployment would
at weight-load time) and ships rsw = 1/(s+eps), s/127 to each core as a tiny
[128, 2] input. That deletes the on-device pass-1 abs-sum (33.5 MB read), the
4-byte AllReduce (~46us of launch+network latency), and the 33.5 MB pass-2
re-read: the device reads each weight chunk ONCE, ternarizes it on arrival,
and starts the matmul stream ~16us in.

GEMM: bf16(stationary xq) x fp8(moving ternary w) at the 216ns/MM bf16 rate.
(fp8 DoubleRow was tried: its 2 multiplies/cell/cycle trips the chip power
limiter - PE clamps to 13/16 clock ~75% of the time - which cancels the 2x
for any exact hi+residual split. bf16 never throttles.)

Schedule: w chunks stream on the scalar ring and ternarize on arrival
(ACT magic-add + 2 DVE ops -> resident w8 fp8). x tiles load + quantize +
transpose on the sync ring. Ramp: k-outer over 8 PSUM-resident groups
(t0,t1 x og0..3) consumes each chunk the moment it lands (the w stream is
DMA-bound, ~40us of unavoidable PE idle). Steady state: t-outer,
k-outer/og-inner; ScalarE applies the fp32 epilogue (gamma*s/127) on
PSUM->SBUF; epilogue stores ride the then-idle scalar ring.
"""

import sys

sys.path.insert(0, "/opt/trn_rl_repo")

import numpy as np

import concourse.bass as bass
import concourse.mybir as mybir
import concourse.tile as tile
import bass_rust
from concourse.bass_utils import run_bass_kernel_spmd

F32 = mybir.dt.float32
BF16 = mybir.dt.bfloat16
FP8 = mybir.dt.float8e4
CMAGIC = 12582912.0  # 2^23 + 2^22: (v + C) - C == round-half-even(v), |v| < 2^22
EPS = 1e-8

N_CORES = 8
B, T, D_IN, D_OUT = 2, 2048, 4096, 16384
TOK = B * T                      # 4096 tokens
OPC = D_OUT // N_CORES           # 2048 out features per core
NTOK = TOK // 128                # 32 token tiles
ND = D_IN // 128                 # 32 contraction tiles
NOG = OPC // 512                 # 4 output groups
DH = D_IN // 2                   # 2048 x staging width
NDH = DH // 128                  # 16 d-tiles per half
XA = 3                           # steady-state x-prep lookahead (tiles)


def _split_multi_waits(nc):
    """This container's walrus build rejects >1 sync wait per instruction, but
    Tile emits multi-wait instructions. Move extra waits onto preceding
    single-wait NoOps on the same engine (identical blocking semantics)."""
    wid = 0
    for f in nc.m.functions:
        for blk in f.blocks:
            insts = list(blk.instructions)
            new = []
            changed = False
            for inst in insts:
                si = inst.sync_info
                if si is not None and len(si.on_wait) > 1:
                    waits = list(si.on_wait)
                    for w in waits[:-1]:
                        nop = mybir.InstNoOp(name=f"WSPLIT-{wid}", ins=[], outs=[])
                        wid += 1
                        nop.engine = inst.engine
                        nop.sync_info = bass_rust.SyncInfo(on_wait=[w], on_update=[])
                        new.append(nop)
                    inst.sync_info = bass_rust.SyncInfo(
                        on_wait=[waits[-1]], on_update=list(si.on_update)
                    )
                    changed = True
                new.append(inst)
            if changed:
                blk.instructions = new


def build_bitlinear_nc():
    nc = bass.Bass("TRN2", target_bir_lowering=False, debug=False,
                   num_devices=N_CORES)
    x_d = nc.dram_tensor("x", [TOK, D_IN], F32, kind="ExternalInput")
    wT_d = nc.dram_tensor("wT", [D_IN, OPC], F32, kind="ExternalInput")
    sc_d = nc.dram_tensor("sc", [128, 2], F32, kind="ExternalInput")
    out_d = nc.dram_tensor("out", [TOK, OPC], F32, kind="ExternalOutput")

    with tile.TileContext(nc, trace_sim=False) as tc:
        with (
            tc.tile_pool(name="w8p", bufs=1) as w8_pool,
            tc.tile_pool(name="w32", bufs=5) as w32_pool,       # streaming w
            tc.tile_pool(name="wtw", bufs=4) as wtw_pool,       # magic-add f32
            tc.tile_pool(name="wmid", bufs=4) as wmid_pool,     # tern bf16
            tc.tile_pool(name="x32", bufs=3) as x32_pool,
            tc.tile_pool(name="xt1", bufs=2) as xt1_pool,
            tc.tile_pool(name="xq16", bufs=2) as xq16_pool,
            tc.tile_pool(name="xqT", bufs=4) as xqT_pool,
            tc.tile_pool(name="outs", bufs=2) as outs_pool,
            tc.tile_pool(name="small", bufs=1) as small,
            tc.tile_pool(name="psum", bufs=2, space="PSUM") as psum_pool,
        ):
            # resident ternary weight, matmul-ready: [d % 128, d // 128, o]
            w8 = w8_pool.tile([128, ND, OPC], FP8, tag="w8", name="w8")
            cmag = small.tile([128, 1], F32)
            nc.gpsimd.memset(cmag[:], CMAGIC)

            # host-computed scales: col0 = 1/(s+eps), col1 = s/127
            scb = small.tile([128, 2], F32)
            nc.scalar.dma_start(scb[:], sc_d[:, :])
            rsw_b = scb[:, 0:1]
            sw127_b = scb[:, 1:2]

            # ---- single weight pass: stream + ternarize on arrival ----
            def tern_k(k):
                wc = w32_pool.tile([128, OPC], F32, tag="w32", name=f"w32_{k}")
                nc.scalar.dma_start(wc[:], wT_d[k * 128:(k + 1) * 128, :])
                for h in range(2):
                    sl = slice(h * 1024, (h + 1) * 1024)
                    tw = wtw_pool.tile([128, 1024], F32, tag="wtw")
                    nc.scalar.activation(tw[:], wc[:, sl],
                                         mybir.ActivationFunctionType.Identity,
                                         bias=cmag[:], scale=rsw_b)
                    tm = wmid_pool.tile([128, 1024], BF16, tag="wmid")
                    nc.vector.tensor_scalar(tm[:], tw[:], -CMAGIC, -1.0,
                                            op0=mybir.AluOpType.add,
                                            op1=mybir.AluOpType.max)
                    nc.vector.tensor_scalar_min(w8[:, k, sl], tm[:], 1.0)

            # ---- x pipeline (sync ring) ----
            xqTs = {}
            evecs = {}
            gams = {}

            def x_load(t):
                xh = []
                for h in range(2):
                    xt = x32_pool.tile([128, DH], F32, tag="x32",
                                       name=f"x_{t}_{h}")
                    nc.sync.dma_start(
                        xt[:], x_d[t * 128:(t + 1) * 128, h * DH:(h + 1) * DH])
                    xh.append(xt)
                return xh

            def emit_evec(t):
                evec = small.tile([128, 1], F32, tag=f"ev{t % 8}", name=f"ev_{t}")
                nc.vector.tensor_tensor(out=evec[:], in0=gams[t], in1=sw127_b,
                                        op=mybir.AluOpType.mult)
                evecs[t] = evec

            def x_compute(t, xh):
                gpart = small.tile([128, 2], F32, tag=f"gp{t % 8}",
                                   name=f"gp_{t}")
                for h in range(2):
                    nc.vector.tensor_reduce(gpart[:, h:h + 1], xh[h][:],
                                            axis=mybir.AxisListType.X,
                                            op=mybir.AluOpType.max,
                                            apply_absolute_value=True)
                gv = small.tile([128, 2], F32, tag=f"gv{t % 8}", name=f"gv_{t}")
                gam, qs = gv[:, 0:1], gv[:, 1:2]
                nc.vector.tensor_reduce(gam, gpart[:], axis=mybir.AxisListType.X,
                                        op=mybir.AluOpType.max)
                nc.vector.tensor_scalar_add(qs, gam, EPS)
                nc.vector.reciprocal(qs, qs)
                nc.vector.tensor_scalar_mul(qs, qs, 127.0)
                gams[t] = gam
                emit_evec(t)

                xqT = xqT_pool.tile([128, ND, 128], BF16, tag="xqT",
                                    name=f"xqT_{t}")
                for h in range(2):
                    xq16 = xq16_pool.tile([128, DH], BF16, tag="xq16")
                    for q in range(2):
                        sl = slice(q * 1024, (q + 1) * 1024)
                        x1 = xt1_pool.tile([128, 1024], F32, tag="xt1")
                        nc.scalar.activation(x1[:], xh[h][:, sl],
                                             mybir.ActivationFunctionType.Identity,
                                             bias=cmag[:], scale=qs)
                        nc.vector.tensor_scalar_add(xq16[:, sl], x1[:], -CMAGIC)
                    nc.sync.dma_start_transpose(
                        out=xqT[:, h * NDH:(h + 1) * NDH, :], in_=xq16[:])
                xqTs[t] = xqT

            def epilogue(t, og, acc):
                ot = outs_pool.tile([128, 512], F32, tag="outs")
                nc.scalar.activation(ot[:], acc[:],
                                     mybir.ActivationFunctionType.Copy,
                                     bias=0.0, scale=evecs[t][:])
                # while the w chunks are still streaming on the scalar ring
                # (ramp + first steady tiles), out-stores must not interleave
                # into them — early tiles store on sync instead
                eng = nc.sync if t < 10 else nc.scalar
                eng.dma_start(
                    out_d[t * 128:(t + 1) * 128, og * 512:(og + 1) * 512], ot[:])

            def mm_tile(t):
                accs = [psum_pool.tile([128, 512], F32, tag=f"acc{og}",
                                       name=f"acc_{t}_{og}")
                        for og in range(NOG)]
                xqT = xqTs[t]
                for k in range(ND):
                    for og in range(NOG):
                        nc.tensor.matmul(
                            accs[og][:], xqT[:, k, :],
                            w8[:, k, og * 512:(og + 1) * 512],
                            start=(k == 0), stop=(k == ND - 1))
                for og in range(NOG):
                    epilogue(t, og, accs[og])

            # ---- schedule ----
            # x t0/t1 first on the sync ring (they gate the ramp), then the
            # ramp: w chunks stream + ternarize, 8 PSUM-resident groups
            # (t0,t1 x og0..3) consume each chunk on arrival. x2..x4 preps are
            # interleaved so their ACT/DVE ops slot between ternarize ops.
            xh0 = x_load(0)
            xh1 = x_load(1)
            x_compute(0, xh0)
            x_compute(1, xh1)

            groups = [(t, og) for t in range(2) for og in range(NOG)]
            accs = {}
            for t, og in groups:
                accs[(t, og)] = psum_pool.tile([128, 512], F32, tag=f"acc{og}",
                                               name=f"acc_{t}_{og}")
            for k in range(ND):
                tern_k(k)
                for t, og in groups:
                    nc.tensor.matmul(accs[(t, og)][:], xqTs[t][:, k, :],
                                     w8[:, k, og * 512:(og + 1) * 512],
                                     start=(k == 0), stop=(k == ND - 1))
                if k == 4:
                    x_compute(2, x_load(2))
                elif k == 8:
                    x_compute(3, x_load(3))
                elif k == 16:
                    x_compute(4, x_load(4))
            for t, og in groups:
                epilogue(t, og, accs[(t, og)])

            # steady state
            for t in range(2, NTOK):
                ta = t + XA
                if 5 <= ta < NTOK:
                    x_compute(ta, x_load(ta))
                mm_tile(t)

    _split_multi_waits(nc)
    return nc


_NC_CACHE = None


def kernel(x: np.ndarray, weight: np.ndarray, _want_profile=False, **_kw):
    global _NC_CACHE
    assert x.shape == (B, T, D_IN) and weight.shape == (D_OUT, D_IN)
    x_flat = np.ascontiguousarray(x.reshape(TOK, D_IN), dtype=np.float32)
    w = np.ascontiguousarray(weight, dtype=np.float32)

    # weight scale on the host (float64 accumulate; the reference's float32
    # pairwise mean differs by ~1e-8 relative — at most a couple of borderline
    # ternary flips across all 67M weights, ~1e-4 output rel err)
    s = np.float64(np.abs(w).mean(dtype=np.float64))
    rsw = np.float32(1.0 / (s + EPS))
    sw127 = np.float32(s / 127.0)
    sc = np.tile(np.array([[rsw, sw127]], dtype=np.float32), (128, 1))

    if _NC_CACHE is None:
        _NC_CACHE = build_bitlinear_nc()
    nc = _NC_CACHE

    in_maps = [
        {"x": x_flat,
         "wT": np.ascontiguousarray(w[c * OPC:(c + 1) * OPC, :].T),
         "sc": sc}
        for c in range(N_CORES)
    ]
    res = run_bass_kernel_spmd(nc, in_maps, list(range(N_CORES)),
                               trace=bool(_want_profile))
    out = np.concatenate([res.results[c]["out"] for c in range(N_CORES)], axis=1)
    out = out.reshape(B, T, D_OUT)
    if _want_profile:
        return out, res
    return out
